# revision 17
# baseline (speedup 1.0000x reference)
"""Trainium2 Bass kernel for CrossFormerAttention-style GNN message passing.

Reference computation (N=50000 nodes, E=1600000 edges, 8 heads x 16 dims):
    Qh = (h_add @ WQ).reshape(N, 8, 16)
    Kh = (h @ WK).reshape(N, 8, 16)
    Vh = (h @ WV).reshape(N, 8, 16)
    score = sum(Kh[src] * Qh[dst], -1)             # [E, 8, 1]
    wV = segment_sum(Vh[src] * score, dst, N)      # [N, 8, 16]
    out = wV / N

Sharding: edges are partitioned by dst range across the 8 cores (6250 nodes
per core) so each core owns a disjoint slice of the output and no collective
is needed.  Within a core, edges are sorted by dst and grouped into 128-edge
subchunks aligned to 128-node blocks; the segment sum becomes a per-block
one-hot matmul accumulated in PSUM.  K/V rows (fused into one 512B bf16 row
per node) are fetched per edge with the Q7 SWDGE dma_gather.  dma_gather
indices are int16, so each subchunk is split host-side into src<32768 ("A",
table base row 0) and src>=32768 ("B", table base row 32768) subchunks.

Perf notes (vs the first working version):
  - per-supergroup metadata (kv idx, q idx, rel) is packed into ONE int16
    upload (rel travels as bf16 bit-patterns; values 0..127/-1 are
    bf16-exact), so phase B has one small HWDGE load per 2048 edges instead
    of three per 1024, and no DVE cast on the critical path.
  - gather/work pools are deep (4/6): the SWDGE gathers were spending
    ~1.6ms waiting on DVE to free 3-deep pool buffers.
  - gathers are issued per SUPERGROUP of 2048 edges (2 compute groups):
    SWDGE emission has a ~1us fixed cost per call plus ~1ns/descriptor on
    HW, so fewer+larger calls halve the Q7 serial time (the phase-B pacing
    engine).
  - phase A processes 4 row-chunks per DMA and stages K|V into full 512B
    rows so every projection-phase DMA moves >=256KB at line rate.
"""

import numpy as np

import concourse.bass as bass
import concourse.mybir as mybir
from concourse import bass_utils
from concourse.bacc import Bacc
from concourse.tile import TileContext

P = 128
N_NODES = 50000
N_EDGES = 1600000
IN_DIM = 128
NUM_HEADS = 8
OUT_DIM = 16
N_CORES = 8
NODES_PER_CORE = N_NODES // N_CORES  # 6250
GROUP_J = 8  # subchunks (of 128 edges) per group -> 1024 edges per group
SUPER = 2  # compute groups per gather supergroup -> 2048 edges per gather set
SG_J = GROUP_J * SUPER  # subchunks per supergroup
I16_BASE = 32768
SUP = 4  # projection chunks per super-tile
MAX_GATHER_SUB = 8  # max subchunks (128 idx each) per dma_gather call
DMA_SCRATCH = 16384  # SWDGE descriptor ring carveout (desc capacity = /16)

F32 = mybir.dt.float32
BF16 = mybir.dt.bfloat16
I16 = mybir.dt.int16


def _ceil_to(x, m):
    return ((x + m - 1) // m) * m


def _to_bf16(a):
    import ml_dtypes

    return np.asarray(a, dtype=np.float32).astype(ml_dtypes.bfloat16)


def shard_edges(src, dst, n_cores=N_CORES, nodes_per_core=NODES_PER_CORE):
    """Partition edges by dst range, sort by dst, split per 128-node block
    into A (src < 32768) and B (src >= 32768) subchunks of 128 edges.
    Schedule (tags / counts) is shared across cores (max over cores) because
    one program runs SPMD on all 8 cores."""
    import ml_dtypes

    src = np.asarray(src).astype(np.int64)
    dst = np.asarray(dst).astype(np.int64)

    order = np.argsort(dst, kind="stable")
    ds = dst[order]
    ss = src[order]

    bounds = np.searchsorted(ds, np.arange(n_cores + 1) * nodes_per_core)
    n_blocks = _ceil_to(nodes_per_core, P) // P  # 49

    # per (core, block) A/B edge lists
    edges = [[None] * n_blocks for _ in range(n_cores)]
    nA = np.zeros((n_cores, n_blocks), dtype=np.int64)
    nB = np.zeros((n_cores, n_blocks), dtype=np.int64)
    for c in range(n_cores):
        sl = slice(bounds[c], bounds[c + 1])
        loc = ds[sl] - c * nodes_per_core
        sc = ss[sl]
        blk = loc // P
        bs = np.searchsorted(blk, np.arange(n_blocks + 1))
        for b in range(n_blocks):
            s2 = slice(bs[b], bs[b + 1])
            l2, s3 = loc[s2], sc[s2]
            a_mask = s3 < I16_BASE
            edges[c][b] = (
                (s3[a_mask], l2[a_mask]),
                (s3[~a_mask], l2[~a_mask]),
            )
            nA[c, b] = int(a_mask.sum())
            nB[c, b] = int((~a_mask).sum())

    subA = ((nA.max(axis=0) + P - 1) // P).astype(np.int64)
    subB = ((nB.max(axis=0) + P - 1) // P).astype(np.int64)
    empty = (subA + subB) == 0
    subA[empty] = 1

    tags = []  # per subchunk: 0=A, 1=B
    blk_of = []
    for b in range(n_blocks):
        tags += [0] * int(subA[b]) + [1] * int(subB[b])
        blk_of += [b] * int(subA[b] + subB[b])
    S = len(tags)
    pad = (-S) % SG_J
    tags += [tags[-1]] * pad
    blk_of += [n_blocks - 1] * pad
    S += pad
    G = S // GROUP_J
    G2 = S // SG_J

    # subchunk start offsets per block for A and B regions
    startA = np.zeros(n_blocks, dtype=np.int64)
    startB = np.zeros(n_blocks, dtype=np.int64)
    off = 0
    for b in range(n_blocks):
        startA[b] = off
        startB[b] = off + subA[b]
        off += subA[b] + subB[b]

    kvidx = np.zeros((n_cores, S * P), dtype=np.int16)
    qidx = np.zeros((n_cores, S * P), dtype=np.int16)
    rel = np.full((n_cores, S * P), -1.0, dtype=np.float32)
    for c in range(n_cores):
        for b in range(n_blocks):
            (sa, la), (sb, lb) = edges[c][b]
            o = int(startA[b]) * P
            kvidx[c, o : o + len(sa)] = sa.astype(np.int16)
            qidx[c, o : o + len(sa)] = la.astype(np.int16)
            rel[c, o : o + len(sa)] = la - b * P
            o = int(startB[b]) * P
            kvidx[c, o : o + len(sb)] = (sb - I16_BASE).astype(np.int16)
            qidx[c, o : o + len(sb)] = lb.astype(np.int16)
            rel[c, o : o + len(sb)] = lb - b * P

    # wrapped int16 index layout: edge i of a 2048-edge supergroup lives at
    # partition i%16, free i//16; replicated x8 over the 128 partitions.
    def wrap16(a):
        w = a.reshape(n_cores, G2, SG_J * 8, 16).transpose(0, 1, 3, 2)
        return np.ascontiguousarray(np.tile(w, (1, 1, 8, 1)))

    kvidx_w = wrap16(kvidx)  # [C, G2, 128, SG_J*8]
    qidx_w = wrap16(qidx)
    # rel upload [C, G2, 128, SG_J]: edge (j, p) -> [g2, p, j], as bf16 bits
    rel_w = np.ascontiguousarray(
        rel.reshape(n_cores, G2, SG_J, P).transpose(0, 1, 3, 2)
    ).astype(ml_dtypes.bfloat16)

    # one packed upload per supergroup:
    # [kvidx(SG_J*8) | qidx(SG_J*8) | rel bf16 bits(SG_J)]
    ein = np.concatenate(
        [kvidx_w, qidx_w, rel_w.view(np.int16)], axis=3
    )  # [C, G2, 128, 272]
    ein = np.ascontiguousarray(ein)

    return {
        "ein": ein,
        "tags": tags,
        "blk_of": blk_of,
        "n_blocks": n_blocks,
        "S": S,
        "G": G,
        "G2": G2,
    }


def build_program(
    *,
    n_nodes_pad,  # KV table rows (mult of 128)
    nodes_core_pad,  # Q table rows (mult of 128)
    n_groups,
    tags,  # per subchunk 0/1
    blk_of,  # per subchunk block id
    scale,
):
    nkv_chunks = n_nodes_pad // P
    nq_chunks = nodes_core_pad // P
    J = GROUP_J
    n_sgroups = n_groups // SUPER
    S = n_groups * J

    first_of = [i == 0 or blk_of[i] != blk_of[i - 1] for i in range(S)]
    last_of = [i == S - 1 or blk_of[i] != blk_of[i + 1] for i in range(S)]

    # per-supergroup maximal same-tag runs [(j0, j1, tag), ...] over SG_J,
    # split at MAX_GATHER_SUB subchunks (descriptor-ring capacity bound)
    runs_of_sgroup = []
    for g in range(n_sgroups):
        runs = []
        j0 = 0
        for j in range(1, SG_J + 1):
            if (
                j == SG_J
                or tags[g * SG_J + j] != tags[g * SG_J + j0]
                or j - j0 >= MAX_GATHER_SUB
            ):
                runs.append((j0, j, tags[g * SG_J + j0]))
                j0 = j
        runs_of_sgroup.append(runs)

    nc = Bacc(num_swdge_queues=4, dynamic_dma_scratch_size=DMA_SCRATCH)

    h_pad = nc.dram_tensor("h_pad", [n_nodes_pad, IN_DIM], F32, kind="ExternalInput")
    hq_pad = nc.dram_tensor(
        "hq_pad", [nodes_core_pad, IN_DIM], F32, kind="ExternalInput"
    )
    wq_d = nc.dram_tensor("wq_d", [IN_DIM, IN_DIM], F32, kind="ExternalInput")
    wk_d = nc.dram_tensor("wk_d", [IN_DIM, IN_DIM], F32, kind="ExternalInput")
    wv_d = nc.dram_tensor("wv_d", [IN_DIM, IN_DIM], F32, kind="ExternalInput")
    ein_d = nc.dram_tensor(
        "ein_d", [n_sgroups, P, 2 * SG_J * 8 + SG_J], I16, kind="ExternalInput"
    )
    wv_out = nc.dram_tensor(
        "wv_out", [(max(blk_of) + 1) * P, IN_DIM], F32, kind="ExternalOutput"
    )

    ident_np = np.eye(P, dtype=np.float32)
    ident_d = nc.inline_tensor(ident_np, name="ident_d")
    iota_np = np.tile(np.arange(P, dtype=np.float32), (P, J))  # [P, J*P]
    iota_d = nc.inline_tensor(_to_bf16(iota_np), name="iota_d")

    with TileContext(nc) as tc:
        with (
            tc.tile_pool(name="const", bufs=1) as constp,
            tc.tile_pool(name="dram", bufs=1, space="DRAM") as dramp,
            tc.tile_pool(name="proj", bufs=3) as projp,
            tc.tile_pool(name="proj_ps", bufs=3, space="PSUM") as projps,
            tc.tile_pool(name="gath", bufs=4) as gathp,
            tc.tile_pool(name="work", bufs=6) as workp,
            tc.tile_pool(name="wv_ps", bufs=2, space="PSUM") as wvps,
            tc.tile_pool(name="outst", bufs=3) as outp,
        ):
            ident = constp.tile([P, P], F32)
            nc.sync.dma_start(ident, ident_d[:])
            iota = constp.tile([P, J * P], BF16)
            nc.sync.dma_start(iota, iota_d[:])

            w_sb = {}
            for name, dram in (("wq", wq_d), ("wk", wk_d), ("wv", wv_d)):
                wf = constp.tile([P, IN_DIM], F32, name=f"{name}_f32")
                nc.sync.dma_start(wf, dram[:])
                wb = constp.tile([P, IN_DIM], BF16, name=f"{name}_bf")
                nc.vector.tensor_copy(wb, wf)
                w_sb[name] = wb

            # fused K|V table: one 512B bf16 row per node
            kv_tab = dramp.tile([n_nodes_pad, 2 * IN_DIM], BF16, name="kv_tab")
            q_tab = dramp.tile([nodes_core_pad, IN_DIM], BF16, name="q_tab")

            # ---- Phase A: projections (SUP row-chunks per DMA) ----
            def project(n_chunks, src_dram, outs, tab, tab_w):
                # outs: list of (w_tile, col0) written into one fused stage
                n_sup = (n_chunks + SUP - 1) // SUP
                for si in range(n_sup):
                    c0 = si * SUP
                    cn = min(SUP, n_chunks - c0)
                    t_h = projp.tile([P, SUP * IN_DIM], F32, name="t_h")
                    nc.sync.dma_start(
                        t_h[:, 0 : cn * IN_DIM].rearrange(
                            "p (c i) -> p c i", i=IN_DIM
                        ),
                        src_dram[c0 * P : (c0 + cn) * P, :].rearrange(
                            "(c p) i -> p c i", p=P
                        ),
                    )
                    stage = projp.tile([P, SUP * tab_w], BF16, name="stage_pr")
                    for c in range(cn):
                        ps_ht = projps.tile([P, P], F32, name="ps_ht")
                        nc.tensor.transpose(
                            ps_ht, t_h[:, c * IN_DIM : (c + 1) * IN_DIM], ident
                        )
                        t_ht = projp.tile([P, P], BF16, name="t_ht")
                        nc.vector.tensor_copy(t_ht, ps_ht)
                        for w_tile, col0 in outs:
                            ps_o = projps.tile([P, IN_DIM], F32, name="ps_o")
                            nc.tensor.matmul(
                                ps_o, lhsT=t_ht, rhs=w_tile, start=True, stop=True
                            )
                            nc.scalar.copy(
                                stage[:, c * tab_w + col0 : c * tab_w + col0 + IN_DIM],
                                ps_o,
                            )
                    nc.sync.dma_start(
                        tab[c0 * P : (c0 + cn) * P, :].rearrange(
                            "(c p) i -> p c i", p=P
                        ),
                        stage[:, 0 : cn * tab_w].rearrange(
                            "p (c i) -> p c i", i=tab_w
                        ),
                    )

            project(
                nkv_chunks,
                h_pad,
                [(w_sb["wk"], 0), (w_sb["wv"], IN_DIM)],
                kv_tab,
                2 * IN_DIM,
            )
            project(nq_chunks, hq_pad, [(w_sb["wq"], 0)], q_tab, IN_DIM)

            kv_lo = kv_tab[0:I16_BASE, :]
            kv_hi = kv_tab[I16_BASE:n_nodes_pad, :]

            # ---- Phase B: edge supergroups ----
            wv_tile = None
            qoff = 2 * SG_J * 8  # rel column offset in ein
            for g2 in range(n_sgroups):
                ein_t = gathp.tile([P, qoff + SG_J], I16, name="ein_t")
                nc.sync.dma_start(ein_t, ein_d[g2])
                idx_t = ein_t[:, 0 : SG_J * 8]
                qidx_t = ein_t[:, SG_J * 8 : 2 * SG_J * 8]

                kv_rows = gathp.tile([P, SG_J * 2 * IN_DIM], BF16, name="kv_rows")
                for (j0, j1, tag) in runs_of_sgroup[g2]:
                    nidx = (j1 - j0) * P
                    nc.gpsimd.dma_gather(
                        out_ap=kv_rows[
                            :, j0 * 2 * IN_DIM : j1 * 2 * IN_DIM
                        ].rearrange("p (c f) -> p c f", f=2 * IN_DIM),
                        in_ap=kv_hi if tag else kv_lo,
                        idxs_ap=idx_t[:, j0 * 8 : j1 * 8],
                        num_idxs=nidx,
                        num_idxs_reg=nidx,
                        elem_size=2 * IN_DIM,
                        queue_num=g2 % 2,
                    )
                q_rows = gathp.tile([P, SG_J * IN_DIM], BF16, name="q_rows")
                for j0 in range(0, SG_J, MAX_GATHER_SUB):
                    j1 = min(j0 + MAX_GATHER_SUB, SG_J)
                    nidx = (j1 - j0) * P
                    nc.gpsimd.dma_gather(
                        out_ap=q_rows[
                            :, j0 * IN_DIM : j1 * IN_DIM
                        ].rearrange("p (c f) -> p c f", f=IN_DIM),
                        in_ap=q_tab[:],
                        idxs_ap=qidx_t[:, j0 * 8 : j1 * 8],
                        num_idxs=nidx,
                        num_idxs_reg=nidx,
                        elem_size=IN_DIM,
                        queue_num=2 + g2 % 2,
                    )

                for cg in range(SUPER):
                    g = g2 * SUPER + cg
                    rel_t = ein_t[
                        :, qoff + cg * J : qoff + (cg + 1) * J
                    ].bitcast(BF16)
                    kvg = kv_rows[
                        :, cg * J * 2 * IN_DIM : (cg + 1) * J * 2 * IN_DIM
                    ]
                    qg = q_rows[:, cg * J * IN_DIM : (cg + 1) * J * IN_DIM]

                    onehot = workp.tile([P, J * P], BF16, name="onehot")
                    nc.vector.tensor_tensor(
                        out=onehot.rearrange("p (j n) -> p j n", j=J),
                        in0=iota.rearrange("p (j n) -> p j n", j=J),
                        in1=rel_t.unsqueeze(-1).to_broadcast([P, J, P]),
                        op=mybir.AluOpType.is_equal,
                    )

                    kv3 = kvg.rearrange("p (j f) -> p j f", f=2 * IN_DIM)
                    k3 = kv3[:, :, 0:IN_DIM]
                    v3 = kv3[:, :, IN_DIM : 2 * IN_DIM]

                    kq = workp.tile([P, J * IN_DIM], BF16, name="kq")
                    nc.vector.tensor_tensor(
                        out=kq.rearrange("p (j f) -> p j f", f=IN_DIM),
                        in0=k3,
                        in1=qg.rearrange("p (j f) -> p j f", f=IN_DIM),
                        op=mybir.AluOpType.mult,
                    )
                    score = workp.tile([P, J * NUM_HEADS], F32, name="score")
                    nc.vector.tensor_reduce(
                        out=score,
                        in_=kq.rearrange("p (jh d) -> p jh d", d=OUT_DIM),
                        axis=mybir.AxisListType.X,
                        op=mybir.AluOpType.add,
                    )
                    score_bf = workp.tile(
                        [P, J * NUM_HEADS], BF16, name="score_bf"
                    )
                    nc.scalar.copy(score_bf, score)
                    msg = workp.tile([P, J * IN_DIM], BF16, name="msg")
                    score_rep = workp.tile([P, J * IN_DIM], BF16, name="score_rep")
                    nc.vector.tensor_copy(
                        score_rep.rearrange("p (jh d) -> p jh d", d=OUT_DIM),
                        score_bf.unsqueeze(-1).to_broadcast(
                            [P, J * NUM_HEADS, OUT_DIM]
                        ),
                    )
                    nc.vector.tensor_tensor(
                        out=msg.rearrange("p (j f) -> p j f", f=IN_DIM),
                        in0=v3,
                        in1=score_rep.rearrange("p (j f) -> p j f", f=IN_DIM),
                        op=mybir.AluOpType.mult,
                    )

                    for j in range(J):
                        sc = g * J + j
                        b = blk_of[sc]
                        if first_of[sc]:
                            wv_tile = wvps.tile([P, IN_DIM], F32, name="wv_tile")
                        nc.tensor.matmul(
                            wv_tile,
                            lhsT=onehot[:, j * P : (j + 1) * P],
                            rhs=msg[:, j * IN_DIM : (j + 1) * IN_DIM],
                            start=first_of[sc],
                            stop=last_of[sc],
                        )
                        if last_of[sc]:
                            stage = outp.tile([P, IN_DIM], F32, name="stage")
                            nc.scalar.mul(stage, wv_tile, scale)
                            nc.sync.dma_start(
                                wv_out[b * P : (b + 1) * P, :], stage
                            )

    nc.finalize()
    return nc


def _make_in_maps(h, h_add, WQ, WK, WV, shard, n_nodes_pad, nodes_core_pad):
    h = np.asarray(h, dtype=np.float32)
    h_add = np.asarray(h_add, dtype=np.float32)
    h_p = np.zeros((n_nodes_pad, IN_DIM), dtype=np.float32)
    h_p[:N_NODES] = h
    in_maps = []
    for c in range(N_CORES):
        hq_p = np.zeros((nodes_core_pad, IN_DIM), dtype=np.float32)
        hq_p[:NODES_PER_CORE] = h_add[
            c * NODES_PER_CORE : (c + 1) * NODES_PER_CORE
        ]
        in_maps.append(
            {
                "h_pad": h_p,
                "hq_pad": hq_p,
                "wq_d": np.asarray(WQ, dtype=np.float32),
                "wk_d": np.asarray(WK, dtype=np.float32),
                "wv_d": np.asarray(WV, dtype=np.float32),
                "ein_d": shard["ein"][c],
            }
        )
    return in_maps


_TRACE = {"trace": False, "last": None, "tmpdir": None}


def kernel(h, h_add, src, dst, WQ, WK, WV):
    shard = shard_edges(src, dst)
    n_nodes_pad = _ceil_to(N_NODES, P)
    nodes_core_pad = _ceil_to(NODES_PER_CORE, P)

    nc = build_program(
        n_nodes_pad=n_nodes_pad,
        nodes_core_pad=nodes_core_pad,
        n_groups=shard["G"],
        tags=shard["tags"],
        blk_of=shard["blk_of"],
        scale=1.0 / N_NODES,
    )
    in_maps = _make_in_maps(h, h_add, WQ, WK, WV, shard, n_nodes_pad, nodes_core_pad)

    res = bass_utils.run_bass_kernel_spmd(
        nc,
        in_maps,
        core_ids=list(range(N_CORES)),
        trace=_TRACE["trace"],
        tmpdir=_TRACE["tmpdir"],
    )
    _TRACE["last"] = res

    out = np.concatenate(
        [np.asarray(res.results[c]["wv_out"])[:NODES_PER_CORE] for c in range(N_CORES)],
        axis=0,
    )
    return out.reshape(N_NODES, NUM_HEADS, OUT_DIM).astype(np.float32)
